# revision 9
# baseline (speedup 1.0000x reference)
"""GNN message-passing (GraphConv + log_softmax head) on 8 Trainium2 cores.

Sharding: nodes by dst across cores; edges bucketed per (core, dst-block of
64, src-range of 25088), counts padded to cross-core maxima (SPMD); per-edge
h_scaled rows fetched via SWDGE dma_gather (32B bf16 rows, 256B-strided DRAM
table built on device); aggregation via one-hot matmuls into PSUM; deg
scaling, @W2+b2, and the 2-layer MLP + log-softmax all on device. Host does
layout only (sort/pad/index arrays/transpose/unshard).
"""

import numpy as np
import ml_dtypes

N_NODES = 100000
D = 16
NC = 8

NP_PAD = 100352              # 128*784, divisible by 8
VC = NP_PAD // NC            # 12544
BLK = 64
NBLK = VC // BLK             # 196
NRANGE = 4
RSZ = NP_PAD // NRANGE       # 25088 = 32*784
TROWS = RSZ + 4              # + zero rows
MLP_PAD = 200192             # 8*25024
MLP_C = MLP_PAD // NC        # 25024
MLP_G = MLP_C // 8           # 3128
GMAX = 1024

_cache = {}


def _emit_dma_gather(gpsimd, out_ap, in_ap, idxs_ap, num_idxs, elem_size,
                     elem_step, queue_num=0):
    """InstDMAGatherAnt emit without the %256 elem-size restriction (the
    non-transpose ucode path only requires the row stride to be 256B)."""
    import concourse.mybir as mybir
    from concourse import ap_utils
    from concourse.bass import MemorySpace, round_up_to_multiple

    assert idxs_ap.dtype == mybir.dt.int16
    assert in_ap.space == MemorySpace.DRAM
    assert idxs_ap.space == MemorySpace.SBUF
    assert out_ap.space == MemorySpace.SBUF
    assert in_ap.dtype == out_ap.dtype
    assert ap_utils.ap_is_contiguous(out_ap.ap[1:])
    assert ap_utils.ap_is_contiguous(idxs_ap.ap[1:])
    assert in_ap.ap[-1][1] == out_ap.ap[-1][1] == elem_size
    assert out_ap.ap[0][1] * out_ap.ap[1][1] == round_up_to_multiple(num_idxs, 128)
    assert in_ap.ap[0][0] == elem_step
    stride_bytes = elem_step * mybir.dt.size(in_ap.dtype)
    assert stride_bytes % 256 == 0 and stride_bytes // 256 < 256
    _in_ap = gpsimd.lower_ap_dma(in_ap, for_custom_bir_dma=True)
    _idxs_ap = gpsimd.lower_ap(idxs_ap)
    _out_ap = gpsimd.lower_ap(out_ap)
    inst = gpsimd.add_instruction(
        mybir.InstDMAGatherAnt(
            name=gpsimd.bass.get_next_instruction_name(),
            ins=[*_in_ap, _idxs_ap,
                 gpsimd.lower_val_access(gpsimd.to_reg(num_idxs))],
            outs=[_out_ap],
            transpose=False, num_idxs=num_idxs, elem_size=elem_size,
            stride_bytes_256=stride_bytes // 256, gen_mode=0,
            single_packet=False, queue_num=queue_num,
            sbuf_tokens_per_rank=0, sbuf_free_dim_per_rank=0,
            sbuf_free_dim_pad_per_rank=0, sbuf_byte_offset=0,
        ))
    return inst


def _build_host(src, dst):
    src = src.astype(np.int64)
    dst = dst.astype(np.int64)
    core = dst // VC
    blk = (dst % VC) // BLK
    rng = src // RSZ
    order = np.lexsort((dst, rng, blk, core))
    s_s, d_s, c_s, b_s, r_s = (src[order], dst[order], core[order],
                               blk[order], rng[order])
    counts = np.zeros((NC, NBLK, NRANGE), dtype=np.int64)
    np.add.at(counts, (c_s, b_s, r_s), 1)
    cell = counts.max(axis=0)
    cell = np.maximum(((cell + 127) // 128) * 128, 128)
    cell_off = np.zeros((NBLK, NRANGE), dtype=np.int64)
    off = 0
    for b in range(NBLK):
        for r in range(NRANGE):
            cell_off[b, r] = off
            off += cell[b, r]
    ncell_tok = int(off)
    idx_local = np.full((NC, ncell_tok), RSZ, dtype=np.int16)
    dstl = np.full((NC, ncell_tok), -1.0, dtype=np.float32)
    gkey = (c_s * NBLK + b_s) * NRANGE + r_s
    gchange = np.empty(len(gkey), dtype=bool)
    gchange[0] = True
    gchange[1:] = gkey[1:] != gkey[:-1]
    gstart = np.where(gchange)[0]
    grank = np.arange(len(gkey)) - np.repeat(
        gstart, np.diff(np.append(gstart, len(gkey))))
    pos = cell_off[b_s, r_s] + grank
    idx_local[c_s, pos] = (s_s % RSZ).astype(np.int16)
    dstl[c_s, pos] = (d_s % BLK).astype(np.float32)
    deg_out = np.bincount(src, minlength=NP_PAD).astype(np.float32)
    deg_in = np.bincount(dst, minlength=NP_PAD).astype(np.float32)
    # per-block gather plans: list of (range, tok_off, n<=GMAX, is_last)
    plans = []
    for b in range(NBLK):
        p = []
        for r in range(NRANGE):
            o, nrem = int(cell_off[b, r]), int(cell[b, r])
            while nrem > 0:
                take = min(GMAX, nrem)
                p.append((r, o, take))
                o += take
                nrem -= take
        plans.append(p)
    return dict(idx_local=idx_local, dstl=dstl, cell=cell, cell_off=cell_off,
                ncell_tok=ncell_tok, deg_out=deg_out, deg_in=deg_in,
                plans=plans)


def _wrap_idx(v):
    n = v.shape[0]
    out = np.zeros((128, n // 16), dtype=np.int16)
    w = v.reshape(-1, 16).T
    for k in range(8):
        out[16 * k:16 * k + 16, :] = w
    return out


def _build_kernel(host):
    import concourse.mybir as mybir
    import concourse.tile as tile
    from concourse import bacc, library_config

    dt = mybir.dt
    ncell_tok = host["ncell_tok"]
    cell = host["cell"]
    cell_off = host["cell_off"]
    plans = host["plans"]

    nc = bacc.Bacc("TRN2", target_bir_lowering=False, debug=False,
                   num_devices=NC)

    def di(name, shape, dty=dt.float32):
        return nc.dram_tensor(name, shape, dty, kind="ExternalInput").ap()

    def do(name, shape, dty=dt.float32):
        return nc.dram_tensor(name, shape, dty, kind="ExternalOutput").ap()

    x_nm = di("x_nm", [128, 784, D])
    degop = di("degop", [128, 784])
    degin_row = di("degin_row", [1, VC])
    idxw = di("idxw", [128, ncell_tok // 16], dt.int16)
    dstlw = di("dstlw", [128, ncell_tok // 128], dt.bfloat16)
    node64 = di("node64", [128, 8 * BLK], dt.bfloat16)
    w2 = di("w2", [D, D])
    b2v = di("b2v", [D, 1])
    xm = di("xm", [128, MLP_G])
    bd1t = di("bd1t", [64, 1])
    sel1 = di("sel1", [16, 128])
    wd1r = di("wd1r", [16, 64])      # Wd1 tiled 8x along free
    mask1 = di("mask1", [128, 64])
    seld = di("seld", [2, 9])
    sel2 = di("sel2", [9, 65])
    mask2 = di("mask2", [65, 8])
    pm1 = di("pm1", [2, 1])          # [-1; 1]
    onesrow = di("onesrow", [1, MLP_G])

    table = nc.dram_tensor("table", [NRANGE * TROWS, 128], dt.bfloat16,
                           kind="Internal").ap()
    out1t = do("out1t", [D, VC])
    lsm0 = do("lsm0", [8, MLP_G])
    lsm1 = do("lsm1", [8, MLP_G])

    with tile.TileContext(nc) as tc:
        with (
            tc.tile_pool(name="sb", bufs=1) as sb,
            tc.tile_pool(name="gp", bufs=6) as gp,
            tc.tile_pool(name="spool", bufs=3) as spool,
            tc.tile_pool(name="ps", bufs=2, space="PSUM") as ps,
            tc.tile_pool(name="ps1", bufs=2, space="PSUM") as ps1,
        ):
            # ---------- persistent small tiles ----------
            t_idx = sb.tile([128, ncell_tok // 16], dt.int16)
            nc.sync.dma_start(t_idx[:], idxw[:])
            t_dstl = sb.tile([128, ncell_tok // 128], dt.bfloat16)
            nc.sync.dma_start(t_dstl[:], dstlw[:])
            t_n64 = sb.tile([128, 8 * BLK], dt.bfloat16)
            nc.sync.dma_start(t_n64[:], node64[:])
            t_w2 = sb.tile([D, D], dt.float32)
            nc.sync.dma_start(t_w2[:], w2[:])
            t_b2 = sb.tile([D, 1], dt.float32)
            nc.sync.dma_start(t_b2[:], b2v[:])
            t_z16bf = sb.tile([128, D], dt.bfloat16)
            nc.vector.memset(t_z16bf[:], 0.0)
            t_ones16 = sb.tile([1, D], dt.float32)
            nc.vector.memset(t_ones16[:], 1.0)

            tview = table.rearrange("(r n) c -> r n c", r=NRANGE)

            # ---------- table prep (transient pool) ----------
            with tc.tile_pool(name="prep", bufs=1) as pp:
                HF = 392
                for hh in range(2):
                    t_x = pp.tile([128, HF, D], dt.float32, tag="px")
                    nc.sync.dma_start(t_x[:], x_nm[:, hh * HF:(hh + 1) * HF])
                    t_dgo = pp.tile([128, HF], dt.float32, tag="pd")
                    nc.sync.dma_start(t_dgo[:], degop[:, hh * HF:(hh + 1) * HF])
                    t_rso = pp.tile([128, HF], dt.float32, tag="pr")
                    nc.scalar.activation(t_rso[:], t_dgo[:],
                                         mybir.ActivationFunctionType.Sqrt)
                    nc.vector.reciprocal(t_rso[:], t_rso[:])
                    nc.vector.tensor_scalar_min(t_rso[:], t_rso[:], 1.0)
                    t_h = pp.tile([128, HF, D], dt.bfloat16, tag="ph")
                    nc.vector.scalar_tensor_tensor(
                        out=t_h[:], in0=t_x[:], scalar=0.0,
                        in1=t_rso[:].rearrange("p (a b) -> p a b", b=1)
                            .to_broadcast([128, HF, D]),
                        op0=mybir.AluOpType.max, op1=mybir.AluOpType.mult)
                    for r in range(NRANGE):
                        nc.gpsimd.dma_start(
                            tview[r, 0:RSZ, 0:D]
                            .rearrange("(p a) b -> p a b", p=32)
                            [:, hh * HF:(hh + 1) * HF],
                            t_h[32 * r:32 * r + 32])
                for r in range(NRANGE):
                    nc.gpsimd.dma_start(tview[r, RSZ:TROWS, 0:D],
                                        t_z16bf[0:TROWS - RSZ, :])

            tc.strict_bb_all_engine_barrier()
            nc.gpsimd.load_library(library_config.mlp)

            # ---------- graph aggregation (rolling 8-block chunks) ----------
            t_aggc = sb.tile([16, 512], dt.float32)
            for b in range(NBLK):
                tok0 = int(cell_off[b, 0])
                ntok_b = int(cell[b].sum())
                nt_b = ntok_b // 128
                t_psb = ps.tile([16, BLK], dt.float32, tag="aggps")
                nc.tensor.matmul(out=t_psb[:], lhsT=t_z16bf[:],
                                 rhs=t_n64[:, 0:BLK], start=True, stop=False)
                t_S = spool.tile([128, nt_b, BLK], dt.bfloat16, tag="S")
                nc.vector.tensor_tensor(
                    out=t_S[:],
                    in0=t_dstl[:, tok0 // 128:tok0 // 128 + nt_b]
                        .rearrange("p (a b) -> p a b", b=1)
                        .to_broadcast([128, nt_b, BLK]),
                    in1=t_n64[:, 0:BLK]
                        .rearrange("p (a b) -> p a b", a=1)
                        .to_broadcast([128, nt_b, BLK]),
                    op=mybir.AluOpType.is_equal)
                tloc = 0
                for (r, o, ntk) in plans[b]:
                    t_g = gp.tile([128, GMAX // 128, D], dt.bfloat16, tag="G")
                    _emit_dma_gather(
                        nc.gpsimd, out_ap=t_g[:, 0:ntk // 128],
                        in_ap=tview[r, :, 0:D],
                        idxs_ap=t_idx[:, o // 16:(o + ntk) // 16],
                        num_idxs=ntk, elem_size=D, elem_step=128)
                    for tt in range(ntk // 128):
                        nc.tensor.matmul(
                            out=t_psb[:], lhsT=t_g[:, tt],
                            rhs=t_S[:, tloc + tt],
                            start=False,
                            stop=(tloc + tt + 1 == nt_b))
                    tloc += ntk // 128
                if b % 8 == 0:
                    c0r = b * BLK
                    wc = min(512, VC - c0r)
                    t_dinc = spool.tile([1, 512], dt.float32, tag="dinc")
                    nc.sync.dma_start(t_dinc[:, 0:wc],
                                      degin_row[:, c0r:c0r + wc])
                    t_psr = ps1.tile([16, 512], dt.float32, tag="big")
                    nc.tensor.matmul(out=t_psr[:, 0:wc], lhsT=t_ones16[:],
                                     rhs=t_dinc[:, 0:wc], start=True, stop=True)
                    t_rsc = spool.tile([16, 512], dt.float32, tag="rsc")
                    nc.scalar.activation(t_rsc[:, 0:wc], t_psr[:, 0:wc],
                                         mybir.ActivationFunctionType.Sqrt)
                    nc.vector.reciprocal(t_rsc[:, 0:wc], t_rsc[:, 0:wc])
                    nc.vector.tensor_scalar_min(t_rsc[:, 0:wc],
                                                t_rsc[:, 0:wc], 1.0)
                cpos = (b % 8) * BLK
                nc.vector.scalar_tensor_tensor(
                    out=t_aggc[:, cpos:cpos + BLK], in0=t_psb[:],
                    scalar=1.0, in1=t_rsc[:, cpos:cpos + BLK],
                    op0=mybir.AluOpType.mult, op1=mybir.AluOpType.mult)
                if b % 8 == 7 or b == NBLK - 1:
                    c0 = (b - (b % 8)) * BLK
                    wc = min(512, VC - c0)
                    t_psw = ps1.tile([16, 512], dt.float32, tag="big")
                    nc.tensor.matmul(out=t_psw[:, 0:wc], lhsT=t_w2[:],
                                     rhs=t_aggc[:, 0:wc], start=True, stop=True)
                    t_o1c = spool.tile([16, 512], dt.float32, tag="o1c")
                    nc.scalar.activation(t_o1c[:, 0:wc], t_psw[:, 0:wc],
                                         mybir.ActivationFunctionType.Identity,
                                         bias=t_b2[:])
                    nc.sync.dma_start(out1t[:, c0:c0 + wc], t_o1c[:, 0:wc])
                    t_aggc = sb.tile([16, 512], dt.float32, tag="t_aggc")

            # ---------- MLP ----------
            t_xm = sb.tile([128, MLP_G], dt.float32)
            nc.sync.dma_start(t_xm[:], xm[:])
            t_sel1 = sb.tile([16, 128], dt.float32)
            nc.sync.dma_start(t_sel1[:], sel1[:])
            t_wd1r = sb.tile([16, 64], dt.float32)
            nc.sync.dma_start(t_wd1r[:], wd1r[:])
            t_mask1 = sb.tile([128, 64], dt.float32)
            nc.sync.dma_start(t_mask1[:], mask1[:])
            t_seld = sb.tile([2, 9], dt.float32)
            nc.sync.dma_start(t_seld[:], seld[:])
            t_sel2 = sb.tile([9, 65], dt.float32)
            nc.sync.dma_start(t_sel2[:], sel2[:])
            t_mask2 = sb.tile([65, 8], dt.float32)
            nc.sync.dma_start(t_mask2[:], mask2[:])
            t_bd1 = sb.tile([64, 1], dt.float32)
            nc.sync.dma_start(t_bd1[:], bd1t[:])
            t_pm1 = sb.tile([2, 1], dt.float32)
            nc.sync.dma_start(t_pm1[:], pm1[:])
            t_one512 = sb.tile([1, 512], dt.float32)
            nc.vector.memset(t_one512[:], 1.0)

            t_psb1 = ps1.tile([128, 64], dt.float32, tag="small")
            nc.tensor.matmul(out=t_psb1[:], lhsT=t_sel1[:], rhs=t_wd1r[:],
                             start=True, stop=True)
            t_bdg1 = sb.tile([128, 64], dt.float32)
            nc.vector.tensor_tensor(out=t_bdg1[:], in0=t_psb1[:],
                                    in1=t_mask1[:], op=mybir.AluOpType.mult)
            t_psw9 = ps1.tile([9, 1], dt.float32, tag="small")
            nc.tensor.matmul(out=t_psw9[:], lhsT=t_seld[:], rhs=t_pm1[:],
                             start=True, stop=True)
            t_wall = sb.tile([9, 8], dt.float32)
            nc.vector.tensor_copy(
                t_wall[:].rearrange("p (a b) -> p a b", b=1),
                t_psw9[:].rearrange("p (a b) -> p a b", b=1)
                .to_broadcast([9, 8, 1]))
            t_psb2 = ps1.tile([65, 8], dt.float32, tag="small")
            nc.tensor.matmul(out=t_psb2[:], lhsT=t_sel2[:], rhs=t_wall[:],
                             start=True, stop=True)
            t_bdg2 = sb.tile([65, 8], dt.float32)
            nc.vector.tensor_tensor(out=t_bdg2[:], in0=t_psb2[:],
                                    in1=t_mask2[:], op=mybir.AluOpType.mult)

            for c0 in range(0, MLP_G, 512):
                w = min(512, MLP_G - c0)
                t_ph1 = ps1.tile([64, 512], dt.float32, tag="big")
                nc.tensor.matmul(out=t_ph1[:, 0:w], lhsT=t_bdg1[:],
                                 rhs=t_xm[:, c0:c0 + w], start=True, stop=True)
                t_h1 = spool.tile([65, 512], dt.float32, tag="h1c")
                nc.scalar.activation(t_h1[0:64, 0:w], t_ph1[:, 0:w],
                                     mybir.ActivationFunctionType.Relu,
                                     bias=t_bd1[:])
                nc.vector.tensor_copy(t_h1[64:65, 0:w], t_one512[:, 0:w])
                t_pd = ps1.tile([8, 512], dt.float32, tag="small")
                nc.tensor.matmul(out=t_pd[:, 0:w], lhsT=t_bdg2[:],
                                 rhs=t_h1[:, 0:w], start=True, stop=True)
                t_ex = spool.tile([8, 512], dt.float32, tag="ex")
                nc.scalar.activation(t_ex[:, 0:w], t_pd[:, 0:w],
                                     mybir.ActivationFunctionType.Exp)
                t_spc = spool.tile([8, 512], dt.float32, tag="spc")
                nc.scalar.activation(t_spc[:, 0:w], t_ex[:, 0:w],
                                     mybir.ActivationFunctionType.Ln,
                                     bias=1.0)
                t_l0c = spool.tile([8, 512], dt.float32, tag="l0c")
                nc.vector.tensor_scalar_mul(t_l0c[:, 0:w], t_spc[:, 0:w], -1.0)
                nc.sync.dma_start(lsm0[:, c0:c0 + w], t_l0c[:, 0:w])
                t_l1c = spool.tile([8, 512], dt.float32, tag="l1c")
                nc.vector.scalar_tensor_tensor(
                    out=t_l1c[:, 0:w], in0=t_pd[:, 0:w], scalar=1.0,
                    in1=t_spc[:, 0:w],
                    op0=mybir.AluOpType.mult, op1=mybir.AluOpType.subtract)
                nc.sync.dma_start(lsm1[:, c0:c0 + w], t_l1c[:, 0:w])

    nc.compile()
    return nc


def kernel(x, external_input, src, dst, W2, b2, Wd1, bd1, Wd2, bd2):
    from concourse.bass_utils import run_bass_kernel_spmd

    x = np.asarray(x, dtype=np.float32)
    external_input = np.asarray(external_input, dtype=np.float32)
    src = np.asarray(src).astype(np.int64)
    dst = np.asarray(dst).astype(np.int64)
    W2 = np.asarray(W2, dtype=np.float32)
    b2 = np.asarray(b2, dtype=np.float32)
    Wd1 = np.asarray(Wd1, dtype=np.float32)
    bd1 = np.asarray(bd1, dtype=np.float32)
    Wd2 = np.asarray(Wd2, dtype=np.float32)
    bd2 = np.asarray(bd2, dtype=np.float32)
    n = x.shape[0]

    host = _build_host(src, dst)
    nc = _build_kernel(host)

    xp = np.zeros((NP_PAD, D), dtype=np.float32)
    xp[:n] = x
    x_nm = np.ascontiguousarray(xp.reshape(128, 784, D))
    degop = np.ascontiguousarray(host["deg_out"].reshape(128, 784))
    node64 = np.broadcast_to(
        np.tile(np.arange(BLK, dtype=np.float32), 8), (128, 8 * BLK)
    ).astype(ml_dtypes.bfloat16).copy()

    sel1 = np.zeros((16, 128), dtype=np.float32)
    for g in range(8):
        for f in range(16):
            sel1[f, 16 * g + f] = 1.0
    wd1r = np.tile(Wd1, (1, 8)).astype(np.float32)
    mask1 = np.zeros((128, 64), dtype=np.float32)
    for g in range(8):
        mask1[16 * g:16 * g + 16, 8 * g:8 * g + 8] = 1.0
    seld = np.concatenate([Wd2.T, bd2[:, None]], axis=1).astype(np.float32)
    sel2 = np.zeros((9, 65), dtype=np.float32)
    for g in range(8):
        for j in range(8):
            sel2[j, 8 * g + j] = 1.0
    sel2[8, 64] = 1.0
    mask2 = np.zeros((65, 8), dtype=np.float32)
    for g in range(8):
        mask2[8 * g:8 * g + 8, g] = 1.0
    mask2[64, :] = 1.0
    bd1t = np.tile(bd1, 8)[:, None].astype(np.float32)
    pm1 = np.array([[-1.0], [1.0]], dtype=np.float32)
    onesrow = np.ones((1, MLP_G), dtype=np.float32)

    catx = np.zeros((MLP_PAD, D), dtype=np.float32)
    catx[:n] = x
    catx[n:2 * n] = external_input

    in_maps = []
    for c in range(NC):
        degin_row = np.ascontiguousarray(
            host["deg_in"][c * VC:(c + 1) * VC][None, :]).astype(np.float32)
        xm_c = np.zeros((128, MLP_G), dtype=np.float32)
        segg = catx[c * MLP_C:(c + 1) * MLP_C].reshape(8, MLP_G, D)
        for g in range(8):
            xm_c[16 * g:16 * g + 16, :] = segg[g].T
        in_maps.append(dict(
            x_nm=x_nm, degop=degop, degin_row=degin_row,
            idxw=_wrap_idx(host["idx_local"][c]),
            dstlw=np.ascontiguousarray(
                host["dstl"][c].reshape(-1, 128).T).astype(ml_dtypes.bfloat16),
            node64=node64, w2=W2, b2v=b2[:, None].astype(np.float32),
            xm=xm_c, bd1t=bd1t, sel1=sel1, wd1r=wd1r, mask1=mask1,
            seld=seld, sel2=sel2, mask2=mask2, pm1=pm1, onesrow=onesrow,
        ))

    import os
    res = run_bass_kernel_spmd(
        nc, in_maps, core_ids=list(range(NC)),
        trace=bool(int(os.environ.get("KERNEL_TRACE", "0"))))
    _cache["exec_time_ns"] = res.exec_time_ns
    _cache["scope_times"] = res.per_core_scope_times

    x_after = np.zeros((NP_PAD, D), dtype=np.float32)
    logits = np.zeros((MLP_PAD, 2), dtype=np.float32)
    for c in range(NC):
        r = res.results[c]
        x_after[c * VC:(c + 1) * VC] = r["out1t"].T
        for g in range(8):
            sl = slice(c * MLP_C + g * MLP_G, c * MLP_C + (g + 1) * MLP_G)
            logits[sl, 0] = r["lsm0"][g]
            logits[sl, 1] = r["lsm1"][g]
    return x_after[:n], logits[:2 * n]
